# revision 23
# baseline (speedup 1.0000x reference)
"""Multi-head attention Trainium2 Bass kernel (v2 — ACT-bound pipeline).

Problem: B=4, T=2048, D=1024, H=16 heads of dim 64 (fp32 in/out).
  qkv = x @ Wqkv.T + bqkv ; per-head attention ; out @ Wo.T + bo

Sharding: 8 cores = 4 batches x 2 head-groups of 8 heads.  Each core
computes its batch's attention for its 8 heads plus the out-projection
restricted to its heads' columns (partial sum); the host adds the two
partial outputs per batch and transposes back.

v2 design notes (vs v0):
  * All matmul operands are bf16 (PSUM accumulation stays fp32).  The
    moving operand streams 2 elem/cycle in bf16, and SBUF/DMA halve.
  * The kernel is restructured to be ACT(exp)-bound: per (pack, query
    chunk of 512, key tile) the PE does one concurrent S-matmul pair
    (heads A/B on disjoint row groups) + two PV matmuls (~0.5us), while
    the ACT engine runs one [128,1024] exp (~1.15us).  PE slack inside
    attention is filled with the next pack's QK generation and, in the
    last pack, with the out-projection for already-finished query
    chunks, so the PE never idles long enough to matter and nothing
    big serializes at the tail.
  * PSUM budget (8 banks): S tiles [128,2,512] double-buffered (4) +
    out A/B accumulators [65,512] (2) + gen/out-proj pool (2).
  * PV lags S by PV_LAG key tiles so the in-order PE stream never
    blocks on an exp or on the previous chunk's normalize.

Per-core layout:
  xT    [1024, 2048] x[b]^T (c-major) bf16
  wqkT  [1024, 4, 256] per head-pack p (2 heads): Q-pack cols | K-pack
  bqk   [128, 4, 2]   per-partition q/k biases matching the pack layout
  wvT   [1024, 512]   V projection (c-major), heads side by side, bf16
  bv    [128, 4]      v-bias laid out like the PV output partitions
  woT   [512, 1024]   Wo columns for this head-group, c_in-major, bf16
  bo    [128, 8]      out bias (zeros on head-group-1 cores)
  yT    [1024, 2048]  output y^T partial (f32)
"""

import sys

sys.path.insert(0, "/opt/trn_rl_repo")

import numpy as np
import ml_dtypes

import concourse.bass as bass  # noqa: F401  (import keeps bass registered)
from concourse import bacc
import concourse.mybir as mybir
import concourse.tile as tile
from concourse.bass_utils import run_bass_kernel_spmd

B, T, D = 4, 2048, 1024
H, HD = 16, 64
P = 128
FP32 = mybir.dt.float32
BF16 = mybir.dt.bfloat16
AF = mybir.ActivationFunctionType
OP = mybir.AluOpType

N_CORES = 8
HPC = 8          # heads per core
NPACK = 4        # head pairs per core
CT = D // P      # 8 contraction tiles over D
KT = T // P      # 16 key tiles
QC = T // 512    # 4 query chunks of 512
SCALE = HD ** -0.5
PV_LAG = 6       # key tiles the PV matmuls trail the S matmuls by

# Every SCHRAU_EVERY-th exp runs on the DVE instead of ACT, via the
# Schraudolph trick emitted straight into bf16 bits:
#   bf16_bits(e^(s*SCALE)) ~= int16(A_S * s + B_S)
# A_S folds the softmax scale into log2 space; B_S = bf16 exponent bias
# shifted into the int16 pattern, minus the standard mantissa-correction
# constant (~0.043 * 2^7) plus 0.5 for the truncating float->int convert.
SCHRAU_EVERY = 4
A_S = float(SCALE * 128.0 / np.log(2.0))
B_S = float(127 * 128 - 5.5 + 0.5)


def build_nc(reps: int = 1, variant: str = "base", dyn: bool = False):
    nc = bacc.Bacc(None, target_bir_lowering=False, debug=False)

    xT_d = nc.dram_tensor("xT", [D, T], BF16, kind="ExternalInput")
    wqkT_d = nc.dram_tensor("wqkT", [D, NPACK, 256], BF16, kind="ExternalInput")
    bqk_d = nc.dram_tensor("bqk", [P, NPACK, 2], FP32, kind="ExternalInput")
    wvT_d = nc.dram_tensor("wvT", [D, HPC * HD], BF16, kind="ExternalInput")
    bv_d = nc.dram_tensor("bv", [P, NPACK], FP32, kind="ExternalInput")
    woT_d = nc.dram_tensor("woT", [NPACK * P, D], BF16, kind="ExternalInput")
    bo_d = nc.dram_tensor("bo", [P, CT], FP32, kind="ExternalInput")
    if dyn:
        nreps_d = nc.dram_tensor("nreps", [1, 1], mybir.dt.int32,
                                 kind="ExternalInput")
    yT_d = nc.dram_tensor("yT", [D, T], FP32, kind="ExternalOutput")

    with tile.TileContext(nc) as tc:
        with (
            tc.tile_pool(name="persist", bufs=1) as persist,
        ):
            ones_col = nc.const_aps.tensor(1.0, [P, 1], BF16)

            # ---- persistent SBUF residents --------------------------------
            xts = persist.tile([P, CT, T], BF16, tag="xts")            # 32 KB/p
            vps = persist.tile([P, KT, HPC * (HD + 1)], BF16, tag="vps")
            attnT = persist.tile([P, NPACK, T], BF16, tag="attnT")     # 16 KB/p
            woTs = persist.tile([P, NPACK, D], BF16, tag="woTs")       # 8 KB/p
            bqks = persist.tile([P, NPACK, 2], FP32, tag="bqks")
            bvs = persist.tile([P, NPACK], FP32, tag="bvs")
            bos = persist.tile([P, CT], FP32, tag="bos")

            for ct in range(CT):
                nc.sync.dma_start(xts[:, ct, :], xT_d[ct * P:(ct + 1) * P, :])
            nc.sync.dma_start(bqks[:], bqk_d[:, :, :])
            nc.sync.dma_start(bvs[:], bv_d[:, :])
            nc.sync.dma_start(bos[:], bo_d[:, :])
            for ci in range(NPACK):
                nc.sync.dma_start(woTs[:, ci, :], woT_d[ci * P:(ci + 1) * P, :])

            if dyn:
                nrt_sb = persist.tile([1, 1], mybir.dt.int32, tag="nrt")
                nc.sync.dma_start(nrt_sb[:], nreps_d[:, :])
                nval = nc.values_load(nrt_sb[0:1, 0:1], min_val=1,
                                      max_val=1 << 20,
                                      skip_runtime_bounds_check=True)
                rep_ctx = tc.For_i(
                    0, nval, 1,
                    hint_engines=(mybir.EngineType.PE,
                                  mybir.EngineType.Activation,
                                  mybir.EngineType.DVE))
            else:
                rep_ctx = None

            import contextlib
            with rep_ctx if rep_ctx is not None else contextlib.nullcontext():
              for _ in range(1 if dyn else reps):
                with (
                    tc.tile_pool(name="wv_pool", bufs=1) as wv_pool,
                    tc.tile_pool(name="wqk_pool", bufs=2) as wqk_pool,
                    tc.tile_pool(name="qkt_pool", bufs=2) as qkt_pool,
                    tc.tile_pool(name="pt_pool", bufs=PV_LAG + 2) as pt_pool,
                    tc.tile_pool(name="rep_pool", bufs=4) as rep_pool,
                    tc.tile_pool(name="ystage_pool", bufs=2) as ystage_pool,
                    tc.tile_pool(name="genpsum", bufs=2, space="PSUM") as genpsum,
                    tc.tile_pool(name="spsum", bufs=2, space="PSUM") as spsum,
                    tc.tile_pool(name="outpsum", bufs=2, space="PSUM") as outpsum,
                ):
                    do_gen = variant not in ("attonly",)
                    do_att = variant not in ("genonly",)

                    # ---- weight loads + V' ones columns -------------------
                    wvs = wv_pool.tile([P, CT, HPC * HD], BF16, tag="wvs")
                    if do_gen:
                        for ct in range(CT):
                            nc.sync.dma_start(
                                wvs[:, ct, :], wvT_d[ct * P:(ct + 1) * P, :])

                    wqk_tiles = {}

                    def load_wqk(p):
                        w = wqk_pool.tile([P, CT, 256], BF16, tag="wqk")
                        for ct in range(CT):
                            nc.sync.dma_start(
                                w[:, ct, :], wqkT_d[ct * P:(ct + 1) * P, p, :])
                        wqk_tiles[p] = w

                    if do_gen:
                        load_wqk(0)

                    vview4 = vps[:].rearrange("p k (h e) -> p k h e", h=HPC)
                    nc.vector.memset(vview4[:, :, :, HD:HD + 1], 1.0)
                    if not do_gen:
                        nc.vector.memset(vview4[:, :, :, 0:HD].bitcast(
                            mybir.dt.uint16), 0)

                    qkt_tiles = {}

                    def alloc_qkt(p):
                        if p not in qkt_tiles:
                            qkt_tiles[p] = qkt_pool.tile([P, 2, T], BF16,
                                                         tag="qkt",
                                                         name=f"qkt{p}")

                    def chain_half(p, jj, tb, half, box):
                        alloc_qkt(p)
                        if half == 0:
                            box["ps"] = genpsum.tile([P, 512], FP32,
                                                     tag="gen", name="gchain")
                        ps = box["ps"]
                        cts = range(4) if half == 0 else range(4, CT)
                        for ct in cts:
                            nc.tensor.matmul(
                                ps[:],
                                wqk_tiles[p][:, ct, jj * P:(jj + 1) * P],
                                xts[:, ct, tb * 512:(tb + 1) * 512],
                                start=(ct == 0), stop=(ct == CT - 1))
                        if half == 1:
                            nc.vector.tensor_scalar_add(
                                qkt_tiles[p][:, jj, tb * 512:(tb + 1) * 512],
                                ps[:], bqks[:, p, jj:jj + 1])

                    def chain_fills(p, jj, tb):
                        box = {}
                        return [lambda: chain_half(p, jj, tb, 0, box),
                                lambda: chain_half(p, jj, tb, 1, box)]

                    def emit_qkgen_chain(p, jj, tb):
                        for f in chain_fills(p, jj, tb):
                            f()

                    def emit_vgen(kt):
                        ps = genpsum.tile([P, HPC * HD], FP32, tag="gen",
                                          name="gv")
                        for ct in range(CT):
                            nc.tensor.matmul(
                                ps[:],
                                xts[:, ct, kt * P:(kt + 1) * P],
                                wvs[:, ct, :],
                                start=(ct == 0), stop=(ct == CT - 1))
                        vview = vps[:, kt, :].rearrange("p (h e) -> p h e",
                                                        h=HPC)
                        nc.vector.tensor_copy(
                            vview[:, :, 0:HD],
                            ps.rearrange("p (h d) -> p h d", h=HPC))

                    def outproj_co(tb, co):
                        ps = genpsum.tile([P, 512], FP32, tag="gen",
                                          name="gop")
                        for ci in range(NPACK):
                            nc.tensor.matmul(
                                ps[:],
                                woTs[:, ci, co * P:(co + 1) * P],
                                attnT[:, ci, tb * 512:(tb + 1) * 512],
                                start=(ci == 0), stop=(ci == NPACK - 1))
                        yst = ystage_pool.tile([P, 512], FP32, tag="yst")
                        nc.vector.tensor_scalar_add(
                            yst[:], ps[:], bos[:, co:co + 1])
                        nc.sync.dma_start(
                            yT_d[co * P:(co + 1) * P,
                                 tb * 512:(tb + 1) * 512],
                            yst[:])

                    def emit_outproj(tb):
                        for co in range(CT):
                            outproj_co(tb, co)

                    if not do_gen:
                        for p in range(NPACK):
                            alloc_qkt(p)
                            nc.vector.memset(
                                qkt_tiles[p][:].bitcast(mybir.dt.uint16), 0)
                    if not do_att:
                        nc.vector.memset(
                            attnT[:].bitcast(mybir.dt.uint16), 0)

                    def normalize(p, qc, outA, outB):
                        q0 = qc * 512
                        for i, outp in ((0, outA), (1, outB)):
                            row0 = i * HD
                            rep = rep_pool.tile([HD, 512], FP32, tag="rep")
                            nc.vector.reciprocal(
                                rep[0:1, :], outp[HD:HD + 1, :])
                            nc.gpsimd.partition_broadcast(
                                rep[:], rep[0:1, :])
                            dst = attnT[row0:row0 + HD, p, q0:q0 + 512]
                            nc.vector.tensor_tensor(
                                dst, outp[0:HD, :], rep[:], OP.mult)
                            nc.vector.tensor_scalar_add(
                                dst, dst, bvs[row0:row0 + HD, p:p + 1])

                    if not do_att:
                        # gen-only: bursts, no pipeline
                        for kt in range(KT):
                            emit_vgen(kt)
                        for p in range(1, NPACK):
                            load_wqk(p)
                            for jj in range(2):
                                for tb in range(QC):
                                    emit_qkgen_chain(p, jj, tb)
                        for tb in range(QC):
                            emit_outproj(tb)
                    else:
                        # ---- flat software-pipelined attention ------------
                        from collections import deque
                        fillq = deque()

                        def pump(budget):
                            k = 0
                            while fillq and k < budget:
                                fillq.popleft()()
                                k += 1

                        out_tiles = {}

                        def emit_pv(p, qc, ktp, pt):
                            if ktp == 0:
                                oA = outpsum.tile([HD + 1, 512], FP32,
                                                  tag="outp", name="outA")
                                oB = outpsum.tile([HD + 1, 512], FP32,
                                                  tag="outp", name="outB")
                                out_tiles[(p, qc)] = (oA, oB)
                            oA, oB = out_tiles[(p, qc)]
                            for i, outp in ((0, oA), (1, oB)):
                                hloc = 2 * p + i
                                nc.tensor.matmul(
                                    outp[:],
                                    vps[:, ktp,
                                        hloc * (HD + 1):(hloc + 1) * (HD + 1)],
                                    pt[:, i * 512:(i + 1) * 512],
                                    start=(ktp == 0), stop=(ktp == KT - 1))
                            if ktp == KT - 1:
                                normalize(p, qc, oA, oB)
                                del out_tiles[(p, qc)]
                                if do_gen and p == NPACK - 1:
                                    for co in range(CT):
                                        fillq.append(
                                            lambda tb=qc, co=co:
                                            outproj_co(tb, co))

                        if do_gen:
                            # lead-in: K and Q chunk-0 chains for pack 0
                            emit_qkgen_chain(0, 1, 0)
                            emit_qkgen_chain(0, 0, 0)
                            # remaining pack-0 gen work, V first (PV needs it)
                            vq = [(lambda kt=kt: emit_vgen(kt))
                                  for kt in range(KT)]
                            cq = []
                            for jj, tb in ((1, 1), (1, 2), (1, 3), (0, 1),
                                           (0, 2), (0, 3)):
                                cq.extend(chain_fills(0, jj, tb))
                            order = [vq[0], vq[1]]
                            vi, ci = 2, 0
                            while vi < KT or ci < len(cq):
                                if ci < len(cq):
                                    order.extend(cq[ci:ci + 2])
                                    ci += 2
                                if vi < KT:
                                    order.extend(vq[vi:vi + 2])
                                    vi += 2
                            fillq.extend(order)
                        else:
                            alloc_qkt(0)

                        # entry e is popped at e + lag(e%16); the lag tapers
                        # from PV_LAG at kt=0 to 2 at kt=15 so each chunk's
                        # normalize lands well before the next chunk's out
                        # tiles are allocated (the PE stream never blocks on
                        # the normalize chain).
                        def pop_at(e):
                            return e + PV_LAG - ((PV_LAG - 2) * (e % KT)) // (
                                KT - 1)

                        pvq = []
                        for gi, (p, qc, kt) in enumerate(
                                (p, qc, kt)
                                for p in range(NPACK)
                                for qc in range(QC)
                                for kt in range(KT)):
                            if do_gen and kt == 0 and qc == 1 and p < NPACK - 1:
                                # enqueue next pack's weights + QK chains
                                fillq.append(lambda p1=p + 1: load_wqk(p1))
                                for jj in range(2):
                                    for tb in range(QC):
                                        fillq.extend(
                                            chain_fills(p + 1, jj, tb))
                            qkt = qkt_tiles[p]
                            q0 = qc * 512
                            sps = spsum.tile([P, 2, 512], FP32, tag="sps")
                            for i in range(2):
                                lo, hi = i * HD, (i + 1) * HD
                                nc.tensor.matmul(
                                    sps[:, i, :],
                                    qkt[lo:hi, 1, kt * P:(kt + 1) * P],
                                    qkt[lo:hi, 0, q0:q0 + 512],
                                    start=True, stop=True)
                            pt = pt_pool.tile([P, 1024], BF16, tag="pt")
                            sps_flat = sps[:].rearrange("p a b -> p (a b)")
                            if SCHRAU_EVERY and gi % SCHRAU_EVERY == 1:
                                nc.vector.tensor_scalar(
                                    pt[:].bitcast(mybir.dt.int16), sps_flat,
                                    A_S, B_S, OP.mult, OP.add)
                            else:
                                nc.scalar.activation(
                                    pt[:], sps_flat, AF.Exp, scale=SCALE)
                            if variant == "dblexp":
                                nc.scalar.activation(
                                    pt[:], sps_flat, AF.Exp, scale=SCALE)
                            pvq.append((gi, p, qc, kt, pt))
                            while pvq and pop_at(pvq[0][0]) <= gi:
                                emit_pv(*pvq.pop(0)[1:])
                            pump(2)
                        for entry in pvq:
                            emit_pv(*entry[1:])
                        pump(len(fillq))
    nc.compile()
    return nc


def _prep_core_inputs(x, Wqkv, bqkv, Wo, bo, core):
    b, g = core // 2, core % 2
    f32 = np.float32
    bf16 = ml_dtypes.bfloat16

    xT = np.ascontiguousarray(x[b].T).astype(bf16)

    wqkT = np.empty((D, NPACK, 256), f32)
    bqk = np.empty((P, NPACK, 2), f32)
    for p in range(NPACK):
        rows_q, rows_k = [], []
        for j in range(2):
            h = 8 * g + 2 * p + j
            rows_q.append(slice(192 * h, 192 * h + 64))
            rows_k.append(slice(192 * h + 64, 192 * h + 128))
        Q2 = np.vstack([Wqkv[rows_q[0]], Wqkv[rows_q[1]]])   # [128, D]
        K2 = np.vstack([Wqkv[rows_k[0]], Wqkv[rows_k[1]]])
        wqkT[:, p, :128] = Q2.T
        wqkT[:, p, 128:] = K2.T
        bqk[:, p, 0] = np.concatenate([bqkv[rows_q[0]], bqkv[rows_q[1]]])
        bqk[:, p, 1] = np.concatenate([bqkv[rows_k[0]], bqkv[rows_k[1]]])

    rows_v = [slice(192 * (8 * g + h) + 128, 192 * (8 * g + h) + 192)
              for h in range(HPC)]
    Wv = np.vstack([Wqkv[r] for r in rows_v])                # [512, D]
    wvT = np.ascontiguousarray(Wv.T).astype(bf16)
    bv = np.empty((P, NPACK), f32)
    for p in range(NPACK):
        bv[:64, p] = bqkv[rows_v[2 * p]]
        bv[64:, p] = bqkv[rows_v[2 * p + 1]]

    woT = np.ascontiguousarray(Wo[:, 512 * g:512 * (g + 1)].T).astype(bf16)
    bo2 = (bo.reshape(CT, P).T.astype(f32).copy() if g == 0
           else np.zeros((P, CT), f32))

    return {
        "xT": xT, "wqkT": wqkT.astype(bf16), "bqk": bqk, "wvT": wvT,
        "bv": bv, "woT": woT, "bo": bo2,
    }


_NC_CACHE = {}


def kernel(x, Wqkv, bqkv, Wo, bo, _reps: int = 1,
           _return_raw: bool = False):
    x = np.asarray(x, np.float32)
    Wqkv = np.asarray(Wqkv, np.float32)
    bqkv = np.asarray(bqkv, np.float32)
    Wo = np.asarray(Wo, np.float32)
    bo = np.asarray(bo, np.float32)

    in_maps = [_prep_core_inputs(x, Wqkv, bqkv, Wo, bo, c)
               for c in range(N_CORES)]

    if _reps not in _NC_CACHE:
        _NC_CACHE[_reps] = build_nc(_reps)
    nc = _NC_CACHE[_reps]

    res = run_bass_kernel_spmd(nc, in_maps, core_ids=list(range(N_CORES)))
    if _return_raw:
        return res

    y = np.empty((B, T, D), np.float32)
    for b in range(B):
        yt = res.results[2 * b]["yT"] + res.results[2 * b + 1]["yT"]
        y[b] = yt.T
    return y


# revision 28
# speedup vs baseline: 1.1701x; 1.1701x over previous
"""Multi-head attention Trainium2 Bass kernel (v2 — ACT-bound pipeline).

Problem: B=4, T=2048, D=1024, H=16 heads of dim 64 (fp32 in/out).
  qkv = x @ Wqkv.T + bqkv ; per-head attention ; out @ Wo.T + bo

Sharding: 8 cores = 4 batches x 2 head-groups of 8 heads.  Each core
computes its batch's attention for its 8 heads plus the out-projection
restricted to its heads' columns (partial sum); the host adds the two
partial outputs per batch and transposes back.

v2 design notes (vs v0):
  * All matmul operands are bf16 (PSUM accumulation stays fp32).  The
    moving operand streams 2 elem/cycle in bf16, and SBUF/DMA halve.
  * The kernel is restructured to be ACT(exp)-bound: per (pack, query
    chunk of 512, key tile) the PE does one concurrent S-matmul pair
    (heads A/B on disjoint row groups) + two PV matmuls (~0.5us), while
    the ACT engine runs one [128,1024] exp (~1.15us).  PE slack inside
    attention is filled with the next pack's QK generation and, in the
    last pack, with the out-projection for already-finished query
    chunks, so the PE never idles long enough to matter and nothing
    big serializes at the tail.
  * PSUM budget (8 banks): S tiles [128,2,512] double-buffered (4) +
    out A/B accumulators [65,512] (2) + gen/out-proj pool (2).
  * PV lags S by PV_LAG key tiles so the in-order PE stream never
    blocks on an exp or on the previous chunk's normalize.

Per-core layout:
  xT    [1024, 2048] x[b]^T (c-major) bf16
  wqkT  [1024, 4, 256] per head-pack p (2 heads): Q-pack cols | K-pack
  bqk   [128, 4, 2]   per-partition q/k biases matching the pack layout
  wvT   [1024, 512]   V projection (c-major), heads side by side, bf16
  bv    [128, 4]      v-bias laid out like the PV output partitions
  woT   [512, 1024]   Wo columns for this head-group, c_in-major, bf16
  bo    [128, 8]      out bias (zeros on head-group-1 cores)
  yT    [1024, 2048]  output y^T partial (f32)
"""

import sys

sys.path.insert(0, "/opt/trn_rl_repo")

import numpy as np
import ml_dtypes

import concourse.bass as bass  # noqa: F401  (import keeps bass registered)
from concourse import bacc
import concourse.mybir as mybir
import concourse.tile as tile
from concourse.bass_utils import run_bass_kernel_spmd

B, T, D = 4, 2048, 1024
H, HD = 16, 64
P = 128
FP32 = mybir.dt.float32
BF16 = mybir.dt.bfloat16
AF = mybir.ActivationFunctionType
OP = mybir.AluOpType

N_CORES = 8
HPC = 8          # heads per core
NPACK = 4        # head pairs per core
CT = D // P      # 8 contraction tiles over D
KT = T // P      # 16 key tiles
QC = T // 512    # 4 query chunks of 512
SCALE = HD ** -0.5
PV_LAG = 6       # key tiles the PV matmuls trail the S matmuls by

# Every SCHRAU_EVERY-th exp runs on the DVE instead of ACT, via the
# Schraudolph trick emitted straight into bf16 bits:
#   bf16_bits(e^(s*SCALE)) ~= int16(A_S * s + B_S)
# A_S folds the softmax scale into log2 space; B_S = bf16 exponent bias
# shifted into the int16 pattern, minus the standard mantissa-correction
# constant (~0.043 * 2^7) plus 0.5 for the truncating float->int convert.
SCHRAU_EVERY = 0  # disabled: DVE queue latency in the sps-release chain
                  # starves the pipeline (544us vs 390us measured)
A_S = float(SCALE * 128.0 / np.log(2.0))
B_S = float(127 * 128 - 5.5 + 0.5)


def build_nc(reps: int = 1, variant: str = "base", dyn: bool = False):
    nc = bacc.Bacc(None, target_bir_lowering=False, debug=False)

    xT_d = nc.dram_tensor("xT", [D, T], BF16, kind="ExternalInput")
    wqkT_d = nc.dram_tensor("wqkT", [D, NPACK, 256], BF16, kind="ExternalInput")
    bqk_d = nc.dram_tensor("bqk", [P, NPACK, 2], FP32, kind="ExternalInput")
    wvT_d = nc.dram_tensor("wvT", [D, HPC * HD], BF16, kind="ExternalInput")
    bv_d = nc.dram_tensor("bv", [P, NPACK], FP32, kind="ExternalInput")
    woT_d = nc.dram_tensor("woT", [NPACK * P, D], BF16, kind="ExternalInput")
    bo_d = nc.dram_tensor("bo", [P, CT], FP32, kind="ExternalInput")
    if dyn:
        nreps_d = nc.dram_tensor("nreps", [1, 1], mybir.dt.int32,
                                 kind="ExternalInput")
    yT_d = nc.dram_tensor("yT", [D, T], FP32, kind="ExternalOutput")

    with tile.TileContext(nc) as tc:
        with (
            tc.tile_pool(name="persist", bufs=1) as persist,
        ):
            ones_col = nc.const_aps.tensor(1.0, [P, 1], BF16)

            # ---- persistent SBUF residents --------------------------------
            xts = persist.tile([P, CT, T], BF16, tag="xts")            # 32 KB/p
            vps = persist.tile([P, KT, HPC * (HD + 1)], BF16, tag="vps")
            attnT = persist.tile([P, NPACK, T], BF16, tag="attnT")     # 16 KB/p
            woTs = persist.tile([P, NPACK, D], BF16, tag="woTs")       # 8 KB/p
            wvs = persist.tile([P, CT, HPC * HD], BF16, tag="wvs")     # 8 KB/p
            wqks = persist.tile([P, NPACK, CT, 256], BF16, tag="wqks")  # 16 KB
            bqks = persist.tile([P, NPACK, 2], FP32, tag="bqks")
            bvs = persist.tile([P, NPACK], FP32, tag="bvs")
            bos = persist.tile([P, CT], FP32, tag="bos")

            for ct in range(CT):
                nc.sync.dma_start(xts[:, ct, :], xT_d[ct * P:(ct + 1) * P, :])
            nc.sync.dma_start(bqks[:], bqk_d[:, :, :])
            nc.sync.dma_start(bvs[:], bv_d[:, :])
            nc.sync.dma_start(bos[:], bo_d[:, :])
            for ci in range(NPACK):
                nc.sync.dma_start(woTs[:, ci, :], woT_d[ci * P:(ci + 1) * P, :])
                for ct in range(CT):
                    nc.sync.dma_start(wqks[:, ci, ct, :],
                                      wqkT_d[ct * P:(ct + 1) * P, ci, :])
            for ct in range(CT):
                nc.sync.dma_start(wvs[:, ct, :], wvT_d[ct * P:(ct + 1) * P, :])

            if dyn:
                nrt_sb = persist.tile([1, 1], mybir.dt.int32, tag="nrt")
                nc.sync.dma_start(nrt_sb[:], nreps_d[:, :])
                nval = nc.values_load(nrt_sb[0:1, 0:1], min_val=1,
                                      max_val=1 << 20,
                                      skip_runtime_bounds_check=True)
                rep_ctx = tc.For_i(
                    0, nval, 1,
                    hint_engines=(mybir.EngineType.PE,
                                  mybir.EngineType.Activation,
                                  mybir.EngineType.DVE))
            else:
                rep_ctx = None

            import contextlib
            with rep_ctx if rep_ctx is not None else contextlib.nullcontext():
              for _ in range(1 if dyn else reps):
                with (
                    tc.tile_pool(name="qkt_pool", bufs=2) as qkt_pool,
                    tc.tile_pool(name="pt_pool", bufs=PV_LAG + 2) as pt_pool,
                    tc.tile_pool(name="rep_pool", bufs=4) as rep_pool,
                    tc.tile_pool(name="ystage_pool", bufs=2) as ystage_pool,
                    tc.tile_pool(name="genpsum", bufs=2, space="PSUM") as genpsum,
                    tc.tile_pool(name="spsum", bufs=2, space="PSUM") as spsum,
                    tc.tile_pool(name="outpsum", bufs=2, space="PSUM") as outpsum,
                ):
                    do_gen = variant not in ("attonly",)
                    do_att = variant not in ("genonly",)

                    wqk_tiles = {p: wqks[:, p, :, :] for p in range(NPACK)}

                    def load_wqk(p):  # weights are persistent now
                        pass

                    vview4 = vps[:].rearrange("p k (h e) -> p k h e", h=HPC)
                    nc.vector.memset(vview4[:, :, :, HD:HD + 1], 1.0)
                    if not do_gen:
                        nc.vector.memset(vview4[:, :, :, 0:HD].bitcast(
                            mybir.dt.uint16), 0)

                    qkt_tiles = {}

                    def alloc_qkt(p):
                        if p not in qkt_tiles:
                            qkt_tiles[p] = qkt_pool.tile([P, 2, T], BF16,
                                                         tag="qkt",
                                                         name=f"qkt{p}")

                    def chain_half(p, jj, tb, half, box):
                        alloc_qkt(p)
                        if half == 0:
                            box["ps"] = genpsum.tile([P, 512], FP32,
                                                     tag="gen", name="gchain")
                        ps = box["ps"]
                        cts = range(4) if half == 0 else range(4, CT)
                        for ct in cts:
                            nc.tensor.matmul(
                                ps[:],
                                wqk_tiles[p][:, ct, jj * P:(jj + 1) * P],
                                xts[:, ct, tb * 512:(tb + 1) * 512],
                                start=(ct == 0), stop=(ct == CT - 1))
                        if half == 1:
                            nc.vector.tensor_scalar_add(
                                qkt_tiles[p][:, jj, tb * 512:(tb + 1) * 512],
                                ps[:], bqks[:, p, jj:jj + 1])

                    def chain_fills(p, jj, tb):
                        box = {}
                        return [lambda: chain_half(p, jj, tb, 0, box),
                                lambda: chain_half(p, jj, tb, 1, box)]

                    def emit_qkgen_chain(p, jj, tb):
                        for f in chain_fills(p, jj, tb):
                            f()

                    def emit_vgen(kt):
                        ps = genpsum.tile([P, HPC * HD], FP32, tag="gen",
                                          name="gv")
                        for ct in range(CT):
                            nc.tensor.matmul(
                                ps[:],
                                xts[:, ct, kt * P:(kt + 1) * P],
                                wvs[:, ct, :],
                                start=(ct == 0), stop=(ct == CT - 1))
                        vview = vps[:, kt, :].rearrange("p (h e) -> p h e",
                                                        h=HPC)
                        nc.vector.tensor_copy(
                            vview[:, :, 0:HD],
                            ps.rearrange("p (h d) -> p h d", h=HPC))

                    def outproj_co(tb, co):
                        ps = genpsum.tile([P, 512], FP32, tag="gen",
                                          name="gop")
                        for ci in range(NPACK):
                            nc.tensor.matmul(
                                ps[:],
                                woTs[:, ci, co * P:(co + 1) * P],
                                attnT[:, ci, tb * 512:(tb + 1) * 512],
                                start=(ci == 0), stop=(ci == NPACK - 1))
                        yst = ystage_pool.tile([P, 512], FP32, tag="yst")
                        nc.vector.tensor_scalar_add(
                            yst[:], ps[:], bos[:, co:co + 1])
                        nc.sync.dma_start(
                            yT_d[co * P:(co + 1) * P,
                                 tb * 512:(tb + 1) * 512],
                            yst[:])

                    def emit_outproj(tb):
                        for co in range(CT):
                            outproj_co(tb, co)

                    if not do_gen:
                        for p in range(NPACK):
                            alloc_qkt(p)
                            nc.vector.memset(
                                qkt_tiles[p][:].bitcast(mybir.dt.uint16), 0)
                    if not do_att:
                        nc.vector.memset(
                            attnT[:].bitcast(mybir.dt.uint16), 0)

                    def normalize(p, qc, outA, outB):
                        q0 = qc * 512
                        for i, outp in ((0, outA), (1, outB)):
                            row0 = i * HD
                            rep = rep_pool.tile([HD, 512], FP32, tag="rep")
                            nc.vector.reciprocal(
                                rep[0:1, :], outp[HD:HD + 1, :])
                            nc.gpsimd.partition_broadcast(
                                rep[:], rep[0:1, :])
                            dst = attnT[row0:row0 + HD, p, q0:q0 + 512]
                            nc.vector.tensor_tensor(
                                dst, outp[0:HD, :], rep[:], OP.mult)
                            nc.vector.tensor_scalar_add(
                                dst, dst, bvs[row0:row0 + HD, p:p + 1])

                    if not do_att:
                        # gen-only: bursts, no pipeline
                        for kt in range(KT):
                            emit_vgen(kt)
                        for p in range(1, NPACK):
                            load_wqk(p)
                            for jj in range(2):
                                for tb in range(QC):
                                    emit_qkgen_chain(p, jj, tb)
                        for tb in range(QC):
                            emit_outproj(tb)
                    else:
                        # ---- flat software-pipelined attention ------------
                        from collections import deque
                        fillq = deque()

                        def pump(budget):
                            k = 0
                            while fillq and k < budget:
                                fillq.popleft()()
                                k += 1

                        out_tiles = {}

                        def emit_pv(p, qc, ktp, pt):
                            if ktp == 0:
                                oA = outpsum.tile([HD + 1, 512], FP32,
                                                  tag="outp", name="outA")
                                oB = outpsum.tile([HD + 1, 512], FP32,
                                                  tag="outp", name="outB")
                                out_tiles[(p, qc)] = (oA, oB)
                            oA, oB = out_tiles[(p, qc)]
                            for i, outp in ((0, oA), (1, oB)):
                                hloc = 2 * p + i
                                nc.tensor.matmul(
                                    outp[:],
                                    vps[:, ktp,
                                        hloc * (HD + 1):(hloc + 1) * (HD + 1)],
                                    pt[:, i * 512:(i + 1) * 512],
                                    start=(ktp == 0), stop=(ktp == KT - 1))
                            if ktp == KT - 1:
                                normalize(p, qc, oA, oB)
                                del out_tiles[(p, qc)]
                                if do_gen and p == NPACK - 1:
                                    for co in range(CT):
                                        fillq.append(
                                            lambda tb=qc, co=co:
                                            outproj_co(tb, co))

                        if do_gen:
                            # lead-in: K and Q chunk-0 chains for pack 0
                            emit_qkgen_chain(0, 1, 0)
                            emit_qkgen_chain(0, 0, 0)
                            # remaining pack-0 gen work, V first (PV needs it)
                            vq = [(lambda kt=kt: emit_vgen(kt))
                                  for kt in range(KT)]
                            cq = []
                            for jj, tb in ((1, 1), (1, 2), (1, 3), (0, 1),
                                           (0, 2), (0, 3)):
                                cq.extend(chain_fills(0, jj, tb))
                            order = [vq[0], vq[1]]
                            vi, ci = 2, 0
                            while vi < KT or ci < len(cq):
                                if ci < len(cq):
                                    order.extend(cq[ci:ci + 2])
                                    ci += 2
                                if vi < KT:
                                    order.extend(vq[vi:vi + 2])
                                    vi += 2
                            fillq.extend(order)
                        else:
                            alloc_qkt(0)

                        # entry e is popped at e + lag(e%16); the lag tapers
                        # from PV_LAG at kt=0 to 2 at kt=15 so each chunk's
                        # normalize lands well before the next chunk's out
                        # tiles are allocated (the PE stream never blocks on
                        # the normalize chain).
                        def pop_at(e):
                            return e + PV_LAG - ((PV_LAG - 2) * (e % KT)) // (
                                KT - 1)

                        pvq = []
                        for gi, (p, qc, kt) in enumerate(
                                (p, qc, kt)
                                for p in range(NPACK)
                                for qc in range(QC)
                                for kt in range(KT)):
                            if do_gen and kt == 0 and qc == 1 and p < NPACK - 1:
                                # enqueue next pack's weights + QK chains
                                fillq.append(lambda p1=p + 1: load_wqk(p1))
                                for jj in range(2):
                                    for tb in range(QC):
                                        fillq.extend(
                                            chain_fills(p + 1, jj, tb))
                            qkt = qkt_tiles[p]
                            q0 = qc * 512
                            sps = spsum.tile([P, 2, 512], FP32, tag="sps")
                            for i in range(2):
                                lo, hi = i * HD, (i + 1) * HD
                                nc.tensor.matmul(
                                    sps[:, i, :],
                                    qkt[lo:hi, 1, kt * P:(kt + 1) * P],
                                    qkt[lo:hi, 0, q0:q0 + 512],
                                    start=True, stop=True)
                            pt = pt_pool.tile([P, 1024], BF16, tag="pt")
                            sps_flat = sps[:].rearrange("p a b -> p (a b)")
                            if SCHRAU_EVERY and gi % SCHRAU_EVERY == 1:
                                nc.vector.tensor_scalar(
                                    pt[:].bitcast(mybir.dt.int16), sps_flat,
                                    A_S, B_S, OP.mult, OP.add)
                            else:
                                nc.scalar.activation(
                                    pt[:], sps_flat, AF.Exp, scale=SCALE)
                            if variant == "dblexp":
                                nc.scalar.activation(
                                    pt[:], sps_flat, AF.Exp, scale=SCALE)
                            pvq.append((gi, p, qc, kt, pt))
                            while pvq and pop_at(pvq[0][0]) <= gi:
                                emit_pv(*pvq.pop(0)[1:])
                            pump(2)
                        for entry in pvq:
                            emit_pv(*entry[1:])
                        pump(len(fillq))
    nc.compile()
    return nc


def _prep_core_inputs(x, Wqkv, bqkv, Wo, bo, core):
    b, g = core // 2, core % 2
    f32 = np.float32
    bf16 = ml_dtypes.bfloat16

    xT = np.ascontiguousarray(x[b].T).astype(bf16)

    wqkT = np.empty((D, NPACK, 256), f32)
    bqk = np.empty((P, NPACK, 2), f32)
    for p in range(NPACK):
        rows_q, rows_k = [], []
        for j in range(2):
            h = 8 * g + 2 * p + j
            rows_q.append(slice(192 * h, 192 * h + 64))
            rows_k.append(slice(192 * h + 64, 192 * h + 128))
        Q2 = np.vstack([Wqkv[rows_q[0]], Wqkv[rows_q[1]]])   # [128, D]
        K2 = np.vstack([Wqkv[rows_k[0]], Wqkv[rows_k[1]]])
        wqkT[:, p, :128] = Q2.T
        wqkT[:, p, 128:] = K2.T
        bqk[:, p, 0] = np.concatenate([bqkv[rows_q[0]], bqkv[rows_q[1]]])
        bqk[:, p, 1] = np.concatenate([bqkv[rows_k[0]], bqkv[rows_k[1]]])

    rows_v = [slice(192 * (8 * g + h) + 128, 192 * (8 * g + h) + 192)
              for h in range(HPC)]
    Wv = np.vstack([Wqkv[r] for r in rows_v])                # [512, D]
    wvT = np.ascontiguousarray(Wv.T).astype(bf16)
    bv = np.empty((P, NPACK), f32)
    for p in range(NPACK):
        bv[:64, p] = bqkv[rows_v[2 * p]]
        bv[64:, p] = bqkv[rows_v[2 * p + 1]]

    woT = np.ascontiguousarray(Wo[:, 512 * g:512 * (g + 1)].T).astype(bf16)
    bo2 = (bo.reshape(CT, P).T.astype(f32).copy() if g == 0
           else np.zeros((P, CT), f32))

    return {
        "xT": xT, "wqkT": wqkT.astype(bf16), "bqk": bqk, "wvT": wvT,
        "bv": bv, "woT": woT, "bo": bo2,
    }


_NC_CACHE = {}


def kernel(x, Wqkv, bqkv, Wo, bo, _reps: int = 1,
           _return_raw: bool = False):
    x = np.asarray(x, np.float32)
    Wqkv = np.asarray(Wqkv, np.float32)
    bqkv = np.asarray(bqkv, np.float32)
    Wo = np.asarray(Wo, np.float32)
    bo = np.asarray(bo, np.float32)

    in_maps = [_prep_core_inputs(x, Wqkv, bqkv, Wo, bo, c)
               for c in range(N_CORES)]

    if _reps not in _NC_CACHE:
        _NC_CACHE[_reps] = build_nc(_reps)
    nc = _NC_CACHE[_reps]

    res = run_bass_kernel_spmd(nc, in_maps, core_ids=list(range(N_CORES)))
    if _return_raw:
        return res

    y = np.empty((B, T, D), np.float32)
    for b in range(B):
        yt = res.results[2 * b]["yT"] + res.results[2 * b + 1]["yT"]
        y[b] = yt.T
    return y
